# revision 24
# baseline (speedup 1.0000x reference)
"""Multi-head causal attention with RoPE on 8 TRN2 NeuronCores.

Problem: x[2,2048,2048] @ {Wq,Wk,Wv}ᵀ -> 16-head causal attention with RoPE
-> @ Woᵀ.  Sharding: core i handles batch i//4 and head-group i%4 (4 heads,
512 of the 2048 projection channels).  Wq/Wk/Wv are row-sliced, Wo is
column-sliced; each core emits a partial yᵀ and the host sums the 4 partials
per batch (the tensor-parallel all-reduce done at unshard time).

Device-side layout (all matmul operands bf16, fp32 PSUM accumulate):
  - host passes xᵀ[h,s] and Wᵀ[h,o] so every matmul contracts over the
    partition dim with zero on-chip transposes
  - startup is DMA-bandwidth-bound (~380GB/s saturated for the first
    ~40us); pieces stream in exact consumption order (xb0/wq hc-lockstep
    across the two HWDGE queues) so the PE never waits on a piece that was
    queued late
  - scores are computed transposed, Sᵀ[k,q] = Kᵀ-chunkᵀ @ Qᵀ, so the exp'd
    attention chunk is directly the lhsT/rhs the PV matmul needs
  - no max-subtraction: scores are ~N(0,1) after the 1/sqrt(128) scale (fused
    into the ACT exp), so exp can't overflow fp32
  - causal masking: block-level by column trimming; the intra-128-block
    triangle is zeroed by a DVE multiply with a 0/1 mask AFTER the exp
    (saves the old -1e30 PE mask matmuls: 8192 PE columns = ~3.4us)
  - softmax denominator: DVE accumulates the exp'd chunk tiles over the
    key-chunk axis (bf16, 2x mode) so the PE only sees a single 512-column
    all-ones matmul per (head, qblock) -- sum over the 128 key partitions
    with M=128, which broadcasts the denominator to every partition for free
    (PE cost is per-column, independent of M).  DVE reciprocal + multiply
    normalizes.
  - PE instruction stream is organized as closed accumulation groups: all
    score matmuls for a (head, qblock) are single start|stop groups, then ONE
    uninterrupted PV accumulation chain.  Interleaving other matmuls into an
    open PSUM accumulation group costs ~80-120ns per transition on TRN2;
    closed-group transitions cost ~10-30ns.
  - tail: output-projection DMAs rotate across the sync/scalar queues; the
    last chains split their PSUM->SBUF copy across ACT/DVE halves with two
    64KB DMAs so the post-matmul drain is short.
"""

import numpy as np
import ml_dtypes

import concourse.bass as bass
import concourse.tile as tile
import concourse.mybir as mybir
from concourse import bacc
from concourse.bass import ts
from concourse.bass_utils import run_bass_kernel_spmd

B, S, H = 2, 2048, 2048
HEADS, HD = 16, 128
NCORES = 8
GH = 4                 # heads per core
GO = GH * HD           # 512 projection channels per core
P = 128
SB = 512               # token-block (free dim of most matmuls)
NSB = S // SB          # 4
HC = H // P            # 16 contraction chunks of the hidden dim
NKC = S // P           # 16 key-token chunks
SCALE = float(HD) ** -0.5

BF16 = mybir.dt.bfloat16
F32 = mybir.dt.float32
EXP = mybir.ActivationFunctionType.Exp

_built = {}


def _build():
    nc = bacc.Bacc(trn_type="TRN2")

    xt = nc.dram_tensor("xt", [H, S], BF16, kind="ExternalInput")
    wqt = nc.dram_tensor("wqt", [H, GO], BF16, kind="ExternalInput")
    wkt = nc.dram_tensor("wkt", [H, GO], BF16, kind="ExternalInput")
    wvt = nc.dram_tensor("wvt", [H, GO], BF16, kind="ExternalInput")
    wot = nc.dram_tensor("wot", [GO, H], BF16, kind="ExternalInput")
    cost = nc.dram_tensor("cost", [P, S], BF16, kind="ExternalInput")
    sint = nc.dram_tensor("sint", [P, S], BF16, kind="ExternalInput")
    # mask01[kk, qq] = 1 where kk <= qq else 0 (causal triangle within a
    # 128x128 diagonal block; applied by DVE multiply after the exp)
    maskt = nc.dram_tensor("maskt", [P, P], BF16, kind="ExternalInput")
    yt = nc.dram_tensor("yt", [H, S], BF16, kind="ExternalOutput")

    xt_r = xt[:].rearrange("(hc p) s -> p hc s", p=P)
    yt_r = yt[:].rearrange("(t p) s -> p t s", p=P)

    with tile.TileContext(nc) as tc:
        with (
            tc.tile_pool(name="const", bufs=1) as const,
            tc.tile_pool(name="xstream", bufs=2) as xpool,
            tc.tile_pool(name="rope", bufs=2) as rpool,
            tc.tile_pool(name="attn", bufs=2) as apool,
            tc.tile_pool(name="den", bufs=2) as dpool,
            tc.tile_pool(name="yout", bufs=8) as ypool,
            tc.tile_pool(name="pacc", bufs=2, space="PSUM") as pacc,
            tc.tile_pool(name="pscore", bufs=2, space="PSUM") as pscore,
            tc.tile_pool(name="pout", bufs=2, space="PSUM") as pout,
        ):
            # ---- constants / persistent tensors ----
            # Startup DMAs fan out over the engine queues so the first
            # projection chain (which consumes xb0 + w_q chunk-by-chunk)
            # isn't serialized behind one queue's issue rate.
            xb0 = xpool.tile([P, HC, SB], BF16, tag="xb")
            w_q = const.tile([P, HC, GO], BF16, tag="wq")
            xt0 = xt_r[:, :, ts(0, SB)]
            wq_r = wqt[:].rearrange("(hc p) o -> p hc o", p=P)
            # the first two hc chunks go as single-chunk pieces so the first
            # projection chain can start sooner; the rest as pairs.
            pieces = [(0, 1), (1, 1), (2, 2), (4, 2), (6, 2), (8, 2), (10, 2),
                      (12, 2), (14, 2)]
            for lo, n in pieces:
                nc.sync.dma_start(xb0[:, lo:lo + n, :], xt0[:, lo:lo + n, :])
                nc.scalar.dma_start(w_q[:, lo:lo + n, :], wq_r[:, lo:lo + n, :])
            w_k = const.tile([P, HC, GO], BF16, tag="wk")
            nc.sync.dma_start(w_k[:], wkt[:].rearrange("(hc p) o -> p hc o", p=P))
            cos_t = const.tile([P, S], BF16, tag="cos")
            nc.scalar.dma_start(cos_t[:], cost[:])
            sin_t = const.tile([P, S], BF16, tag="sin")
            nc.scalar.dma_start(sin_t[:], sint[:])
            mask_t = const.tile([P, P], BF16, tag="mask")
            nc.scalar.dma_start(mask_t[:], maskt[:])
            w_v = const.tile([P, HC, GO], BF16, tag="wv")
            nc.sync.dma_start(w_v[:], wvt[:].rearrange("(hc p) o -> p hc o", p=P))
            xbs = [xb0]
            xb1 = xpool.tile([P, HC, SB], BF16, tag="xb")
            nc.sync.dma_start(xb1[:], xt_r[:, :, ts(1, SB)])
            xbs.append(xb1)

            q_t = const.tile([P, GH, S], BF16, tag="qt")
            k_t = const.tile([P, GH, S], BF16, tag="kt")
            v_t = const.tile([P, NKC, GO], BF16, tag="vt")
            out_t = const.tile([P, GH, S], BF16, tag="ot")
            ones_t = const.tile([P, P], BF16, tag="ones")
            nc.gpsimd.memset(ones_t[:], 1.0)

            MM_NS = 0.4167e-3  # PE stream: us per column

            # ---- emission generators.  Every yield is a CLOSED PE group
            # boundary; the yielded value is the quantum's PE time (us). ----

            def proj_sb(sb, xb):
                """One token-block of Q/K (with RoPE) and V projections."""
                for w_t, dest in ((w_q, q_t), (w_k, k_t)):
                    for h in range(GH):
                        ps = pacc.tile([P, SB], F32, tag="pp")
                        for hc in range(HC):
                            nc.tensor.matmul(
                                ps[:], w_t[:, hc, ts(h, P)], xb[:, hc, :],
                                start=(hc == 0), stop=(hc == HC - 1),
                            )
                        raw = dest[:, h, ts(sb, SB)]
                        nc.scalar.copy(raw, ps[:])
                        # RoPE: rot = raw*cos + shift(raw)*sin_signed
                        tmp = rpool.tile([P, SB], BF16, tag="sh")
                        nc.sync.dma_start(tmp[0:64, :], raw[64:128, :])
                        nc.sync.dma_start(tmp[64:128, :], raw[0:64, :])
                        tmp2 = rpool.tile([P, SB], BF16, tag="sp")
                        nc.vector.tensor_mul(tmp2[:], tmp[:], sin_t[:, ts(sb, SB)])
                        nc.vector.tensor_mul(raw, raw, cos_t[:, ts(sb, SB)])
                        nc.vector.tensor_add(raw, raw, tmp2[:])
                        yield 3.41
                for j in range(SB // P):
                    ps = pacc.tile([P, GO], F32, tag="pp")
                    for hc in range(HC):
                        nc.tensor.matmul(
                            ps[:], xb[:, hc, ts(j, P)], w_v[:, hc, :],
                            start=(hc == 0), stop=(hc == HC - 1),
                        )
                    nc.scalar.copy(v_t[:, sb * (SB // P) + j, :], ps[:])
                    yield 3.41

            def attn_block(b):
                """Attention for one 512-query block.

                Per head: score matmuls as closed start|stop groups (chunk
                pairs share a 2-bank psum tile and one paired ACT exp), the
                denominator accumulating on DVE behind the exps, then one
                uninterrupted PV accumulation chain.

                For the small blocks (b<2) several heads pack into one `at`
                tile so ALL scores run before the first PV -- the exps get
                multiple score-groups of slack.  Big blocks pipeline PV one
                head behind the scores."""
                nchunks = 4 * (b + 1)
                noff = 4 * b
                hpt = {0: 4, 1: 2}.get(b, 1)   # heads per at tile
                pend = []                       # (h, at, base, den1)

                def emit_pv(h, at, base, den1, last=False):
                    po = pout.tile([P, SB], F32, tag="po")
                    for c in range(nchunks):
                        j = c - noff
                        qlo = 128 * j if j > 0 else 0
                        nc.tensor.matmul(
                            po[:, qlo:], v_t[:, c, ts(h, P)],
                            at[:, base + c, qlo:],
                            start=(c == 0), stop=(c == nchunks - 1),
                        )
                    # denominator: one 512-col all-ones matmul sums the DVE
                    # chunk-fold over the 128 key partitions AND broadcasts
                    # it to all partitions (cost is per-column, M-free).
                    # Emitted after the PV chain so the DVE fold has a full
                    # chain's slack before the PE reaches it.  The last pop
                    # of the last block borrows the (then idle) score pool
                    # so the first outproj(3) psum tile doesn't queue behind
                    # the denominator's DVE read.
                    if last and b == NSB - 1:
                        psd = pscore.tile([P, 2, SB], F32, tag="ps")
                        pd = psd[:, 0, :]
                    else:
                        pdt = pacc.tile([P, SB], F32, tag="pp")
                        pd = pdt[:]
                    nc.tensor.matmul(pd, ones_t[:], den1, start=True, stop=True)
                    rec = dpool.tile([P, SB], F32, tag="rec")
                    nc.vector.reciprocal_approx_fast(rec[:], pd)
                    nc.vector.tensor_mul(out_t[:, h, ts(b, SB)], po[:], rec[:])

                at = den = None
                for h in range(GH):
                    if h % hpt == 0:
                        at = apool.tile([P, NKC, SB], BF16, tag="at")
                    if h % 2 == 0 and b < 2:
                        den = dpool.tile([P, 2, SB], BF16, tag="den")
                    elif b >= 2:
                        den = dpool.tile([P, 2, SB], BF16, tag="den")
                    base = (h % hpt) * nchunks
                    # b<2: each head owns ONE half of a shared den tile;
                    # b>=2: a full tile (pairs use both halves, then fold).
                    den1 = den[:, h % 2, :] if b < 2 else den[:, 0, :]
                    # off-diagonal chunk pairs
                    for i in range(noff // 2):
                        psc = pscore.tile([P, 2, SB], F32, tag="ps")
                        for m in (0, 1):
                            c = 2 * i + m
                            nc.tensor.matmul(
                                psc[:, m, :], k_t[:, h, ts(c, P)],
                                q_t[:, h, ts(b, SB)],
                                start=True, stop=True,
                            )
                        s = base + 2 * i
                        nc.scalar.activation(
                            at[:, s:s + 2, :], psc[:, :, :], EXP, scale=SCALE
                        )
                        if b >= 2:
                            if i == 0:
                                nc.vector.tensor_copy(den[:], at[:, s:s + 2, :])
                            else:
                                nc.vector.tensor_add(
                                    den[:], den[:], at[:, s:s + 2, :]
                                )
                        elif i == 0:
                            nc.vector.tensor_add(
                                den1, at[:, s, :], at[:, s + 1, :]
                            )
                        else:
                            nc.vector.tensor_add(den1, den1, at[:, s, :])
                            nc.vector.tensor_add(den1, den1, at[:, s + 1, :])
                        yield 0.43
                    # diagonal chunks, two per psum pair tile; the causal
                    # triangle is zeroed by a DVE mask multiply after exp
                    for i in range(2):
                        psc = pscore.tile([P, 2, SB], F32, tag="ps")
                        for m in (0, 1):
                            j = 2 * i + m
                            qlo = 128 * j
                            n = SB - qlo
                            nc.tensor.matmul(
                                psc[:, m, 0:n], k_t[:, h, ts(noff + j, P)],
                                q_t[:, h, b * SB + qlo:(b + 1) * SB],
                                start=True, stop=True,
                            )
                        for m in (0, 1):
                            j = 2 * i + m
                            qlo = 128 * j
                            n = SB - qlo
                            s = base + noff + j
                            nc.scalar.activation(
                                at[:, s, qlo:], psc[:, m, 0:n], EXP,
                                scale=SCALE,
                            )
                            nc.vector.tensor_mul(
                                at[:, s, qlo:qlo + P],
                                at[:, s, qlo:qlo + P], mask_t[:],
                            )
                            if b == 0 and j == 0:
                                nc.vector.tensor_copy(den1, at[:, s, :])
                            else:
                                dst = den1 if b < 2 else den[:, 0, :]
                                nc.vector.tensor_add(
                                    dst[:, qlo:] if b < 2 else den[:, 0, qlo:],
                                    dst[:, qlo:] if b < 2 else den[:, 0, qlo:],
                                    at[:, s, qlo:],
                                )
                        yield 0.37
                    if b >= 2:
                        # fold the two accumulator halves on DVE; overlaps
                        # the next head's scores so the PE den matmul in
                        # emit_pv never waits on it.
                        nc.vector.tensor_add(
                            den[:, 0, :], den[:, 0, :], den[:, 1, :]
                        )
                        den1 = den[:, 0, :]
                    pend.append((h, at, base, den1))
                    while len(pend) > (GH if b < 2 else 1):
                        emit_pv(*pend.pop(0))
                        yield (1280 + 2048 * b) * MM_NS
                while pend:
                    emit_pv(*pend.pop(0), last=(not pend))
                    yield (1280 + 2048 * b) * MM_NS

            NT = H // P
            OQ = (nc.sync, nc.scalar)

            def outproj_block(ob, tail=False):
                """Output projection of one query block; yields per closed
                4-matmul chain.  Output DMAs rotate across the sync/scalar
                queues.  While interleaved with attention the PSUM->SBUF
                staging copies stay on DVE (keeping ACT free for the
                latency-critical exps); in the tail they alternate, and the
                last chains split the copy across ACT/DVE halves with two
                64KB DMAs so the post-matmul drain is short."""
                for nt in range(NT):
                    if tail and nt == NT - 1:
                        # final chain: two 256-col sub-chains so the last
                        # copy+DMA starts half a chain earlier
                        HB = SB // 2
                        for half in range(2):
                            co = ob * SB + half * HB
                            pyt = pacc.tile([P, SB], F32, tag="pp")
                            for oc in range(GH):
                                nc.tensor.matmul(
                                    pyt[:, 0:HB], w_o[:, oc, ts(nt, P)],
                                    out_t[:, oc, co:co + HB],
                                    start=(oc == 0), stop=(oc == GH - 1),
                                )
                            ysb = ypool.tile([P, SB], BF16, tag="ysb")
                            if half == 0:
                                nc.scalar.copy(ysb[:, 0:HB], pyt[:, 0:HB])
                            else:
                                nc.vector.tensor_copy(ysb[:, 0:HB],
                                                      pyt[:, 0:HB])
                            OQ[half].dma_start(
                                yt_r[:, nt, co:co + HB], ysb[:, 0:HB]
                            )
                        yield 0.85
                        continue
                    pyt = pacc.tile([P, SB], F32, tag="pp")
                    for oc in range(GH):
                        nc.tensor.matmul(
                            pyt[:], w_o[:, oc, ts(nt, P)],
                            out_t[:, oc, ts(ob, SB)],
                            start=(oc == 0), stop=(oc == GH - 1),
                        )
                    ysb = ypool.tile([P, SB], BF16, tag="ysb")
                    if tail and nt >= NT - 4:
                        HB = SB // 2
                        nc.scalar.copy(ysb[:, 0:HB], pyt[:, 0:HB])
                        nc.vector.tensor_copy(ysb[:, HB:SB], pyt[:, HB:SB])
                        OQ[nt % 2].dma_start(
                            yt_r[:, nt, ob * SB:ob * SB + HB], ysb[:, 0:HB]
                        )
                        OQ[(nt + 1) % 2].dma_start(
                            yt_r[:, nt, ob * SB + HB:(ob + 1) * SB],
                            ysb[:, HB:SB],
                        )
                    else:
                        if tail and nt % 2 == 1:
                            nc.scalar.copy(ysb[:], pyt[:])
                        else:
                            nc.vector.tensor_copy(ysb[:], pyt[:])
                        OQ[nt % 2].dma_start(yt_r[:, nt, ts(ob, SB)], ysb[:])
                    yield 0.85

            def drain(gen):
                for _ in gen:
                    pass

            def chain2(*gens):
                for g in gens:
                    yield from g

            def interleave(primary, filler, ratio, drain_rest=True):
                """Emit primary; between its quanta emit filler quanta so
                filler-PE-time ~= ratio * primary-PE-time."""
                acc = 0.0
                done = False
                for wp in primary:
                    acc += ratio * (wp or 1.0)
                    while not done and acc > 0:
                        wf = next(filler, None)
                        if wf is None:
                            done = True
                        else:
                            acc -= wf or 1.0
                if drain_rest and not done:
                    drain(filler)

            # ---- drive ----
            drain(proj_sb(0, xbs[0]))

            def proj_one(sb):
                # prefetch the NEXT x block after the first chain of this
                # one, so its (single-queue) DMA lands well before use.
                first = True
                for w in proj_sb(sb, xbs[sb]):
                    yield w
                    if first:
                        first = False
                        if sb + 1 < NSB and len(xbs) == sb + 1:
                            xbn = xpool.tile([P, HC, SB], BF16, tag="xb")
                            nc.sync.dma_start(xbn[:], xt_r[:, :, ts(sb + 1, SB)])
                            xbs.append(xbn)

            interleave(proj_one(1), attn_block(0), 5.2 / 41.0)
            interleave(proj_one(2), attn_block(1), 12.1 / 41.0)
            # Wo reuses an x-stream slot (same bytes); loaded once proj(2)
            # has consumed xb2, well before outproj(0) needs it.
            w_o = xpool.tile([P, GH, H], BF16, tag="xb")
            nc.sync.dma_start(w_o[:], wot[:].rearrange("(oc p) n -> p oc n", p=P))
            interleave(proj_one(3), attn_block(2), 18.8 / 41.0)
            # ratio slightly under-fills so a couple of outproj(2) chains
            # remain after attn(3) to cover its DVE normalize tail
            interleave(
                attn_block(3),
                chain2(outproj_block(0), outproj_block(1), outproj_block(2)),
                1.5,
            )
            drain(outproj_block(NSB - 1, tail=True))

    nc.compile()
    return nc


def _get_nc():
    if "nc" not in _built:
        _built["nc"] = _build()
    return _built["nc"]


def _host_inputs(x, Wq, Wk, Wv, Wo):
    bf = ml_dtypes.bfloat16
    inv = 1.0 / (10000.0 ** (np.arange(0, HD, 2, dtype=np.float64) / HD))
    t = np.arange(S, dtype=np.float64)
    fr = np.outer(t, inv)                       # [S, 64]
    cos = np.cos(fr)
    sin = np.sin(fr)
    cosT = np.concatenate([cos, cos], axis=1).T.astype(bf)      # [128, S]
    sinT = np.concatenate([-sin, sin], axis=1).T.astype(bf)     # signed
    a = np.arange(P)
    mask01 = (a[:, None] <= a[None, :]).astype(bf)

    in_maps = []
    for core in range(NCORES):
        b, g = divmod(core, GH)
        sl = slice(GO * g, GO * (g + 1))
        in_maps.append({
            "xt": np.ascontiguousarray(x[b].T).astype(bf),
            "wqt": np.ascontiguousarray(Wq[sl, :].T).astype(bf),
            "wkt": np.ascontiguousarray(Wk[sl, :].T).astype(bf),
            "wvt": np.ascontiguousarray(Wv[sl, :].T).astype(bf),
            "wot": np.ascontiguousarray(Wo[:, sl].T).astype(bf),
            "cost": cosT.copy(),
            "sint": sinT.copy(),
            "maskt": mask01.copy(),
        })
    return in_maps


def kernel(x, Wq, Wk, Wv, Wo, _trace=False):
    x = np.asarray(x, dtype=np.float32)
    Wq = np.asarray(Wq, dtype=np.float32)
    Wk = np.asarray(Wk, dtype=np.float32)
    Wv = np.asarray(Wv, dtype=np.float32)
    Wo = np.asarray(Wo, dtype=np.float32)

    nc = _get_nc()
    in_maps = _host_inputs(x, Wq, Wk, Wv, Wo)
    res = run_bass_kernel_spmd(
        nc, in_maps, core_ids=list(range(NCORES)), trace=_trace
    )
    if _trace:
        _built["last_result"] = res

    y = np.zeros((B, S, H), dtype=np.float32)
    for core in range(NCORES):
        b = core // GH
        y[b] += res.results[core]["yt"].T.astype(np.float32)
    return y


# revision 25
# speedup vs baseline: 1.0033x; 1.0033x over previous
"""Multi-head causal attention with RoPE on 8 TRN2 NeuronCores.

Problem: x[2,2048,2048] @ {Wq,Wk,Wv}ᵀ -> 16-head causal attention with RoPE
-> @ Woᵀ.  Sharding: core i handles batch i//4 and head-group i%4 (4 heads,
512 of the 2048 projection channels).  Wq/Wk/Wv are row-sliced, Wo is
column-sliced; each core emits a partial yᵀ and the host sums the 4 partials
per batch (the tensor-parallel all-reduce done at unshard time).

Device-side layout (all matmul operands bf16, fp32 PSUM accumulate):
  - host passes xᵀ[h,s] and Wᵀ[h,o] so every matmul contracts over the
    partition dim with zero on-chip transposes
  - startup is DMA-bandwidth-bound (~380GB/s saturated for the first
    ~40us); pieces stream in exact consumption order (xb0/wq hc-lockstep
    across the two HWDGE queues) so the PE never waits on a piece that was
    queued late
  - scores are computed transposed, Sᵀ[k,q] = Kᵀ-chunkᵀ @ Qᵀ, so the exp'd
    attention chunk is directly the lhsT/rhs the PV matmul needs
  - no max-subtraction: scores are ~N(0,1) after the 1/sqrt(128) scale (fused
    into the ACT exp), so exp can't overflow fp32
  - causal masking: block-level by column trimming; the intra-128-block
    triangle is zeroed by a DVE multiply with a 0/1 mask AFTER the exp
    (saves the old -1e30 PE mask matmuls: 8192 PE columns = ~3.4us)
  - softmax denominator: DVE accumulates the exp'd chunk tiles over the
    key-chunk axis (bf16, 2x mode) so the PE only sees a single 512-column
    all-ones matmul per (head, qblock) -- sum over the 128 key partitions
    with M=128, which broadcasts the denominator to every partition for free
    (PE cost is per-column, independent of M).  DVE reciprocal + multiply
    normalizes.
  - PE instruction stream is organized as closed accumulation groups: all
    score matmuls for a (head, qblock) are single start|stop groups, then ONE
    uninterrupted PV accumulation chain.  Interleaving other matmuls into an
    open PSUM accumulation group costs ~80-120ns per transition on TRN2;
    closed-group transitions cost ~10-30ns.
  - tail: output-projection DMAs rotate across the sync/scalar queues; the
    last chains split their PSUM->SBUF copy across ACT/DVE halves with two
    64KB DMAs so the post-matmul drain is short.
"""

import numpy as np
import ml_dtypes

import concourse.bass as bass
import concourse.tile as tile
import concourse.mybir as mybir
from concourse import bacc
from concourse.bass import ts
from concourse.bass_utils import run_bass_kernel_spmd

B, S, H = 2, 2048, 2048
HEADS, HD = 16, 128
NCORES = 8
GH = 4                 # heads per core
GO = GH * HD           # 512 projection channels per core
P = 128
SB = 512               # token-block (free dim of most matmuls)
NSB = S // SB          # 4
HC = H // P            # 16 contraction chunks of the hidden dim
NKC = S // P           # 16 key-token chunks
SCALE = float(HD) ** -0.5

BF16 = mybir.dt.bfloat16
F32 = mybir.dt.float32
EXP = mybir.ActivationFunctionType.Exp

_built = {}


def _build():
    nc = bacc.Bacc(trn_type="TRN2")

    xt = nc.dram_tensor("xt", [H, S], BF16, kind="ExternalInput")
    wqt = nc.dram_tensor("wqt", [H, GO], BF16, kind="ExternalInput")
    wkt = nc.dram_tensor("wkt", [H, GO], BF16, kind="ExternalInput")
    wvt = nc.dram_tensor("wvt", [H, GO], BF16, kind="ExternalInput")
    wot = nc.dram_tensor("wot", [GO, H], BF16, kind="ExternalInput")
    cost = nc.dram_tensor("cost", [P, S], BF16, kind="ExternalInput")
    sint = nc.dram_tensor("sint", [P, S], BF16, kind="ExternalInput")
    # mask01[kk, qq] = 1 where kk <= qq else 0 (causal triangle within a
    # 128x128 diagonal block; applied by DVE multiply after the exp)
    maskt = nc.dram_tensor("maskt", [P, P], BF16, kind="ExternalInput")
    yt = nc.dram_tensor("yt", [H, S], BF16, kind="ExternalOutput")

    xt_r = xt[:].rearrange("(hc p) s -> p hc s", p=P)
    yt_r = yt[:].rearrange("(t p) s -> p t s", p=P)

    with tile.TileContext(nc) as tc:
        with (
            tc.tile_pool(name="const", bufs=1) as const,
            tc.tile_pool(name="xstream", bufs=2) as xpool,
            tc.tile_pool(name="rope", bufs=2) as rpool,
            tc.tile_pool(name="attn", bufs=2) as apool,
            tc.tile_pool(name="den", bufs=2) as dpool,
            tc.tile_pool(name="yout", bufs=8) as ypool,
            tc.tile_pool(name="pacc", bufs=2, space="PSUM") as pacc,
            tc.tile_pool(name="pscore", bufs=2, space="PSUM") as pscore,
            tc.tile_pool(name="pout", bufs=2, space="PSUM") as pout,
        ):
            # ---- constants / persistent tensors ----
            # Startup DMAs fan out over the engine queues so the first
            # projection chain (which consumes xb0 + w_q chunk-by-chunk)
            # isn't serialized behind one queue's issue rate.
            xb0 = xpool.tile([P, HC, SB], BF16, tag="xb")
            w_q = const.tile([P, HC, GO], BF16, tag="wq")
            xt0 = xt_r[:, :, ts(0, SB)]
            wq_r = wqt[:].rearrange("(hc p) o -> p hc o", p=P)
            # the first two hc chunks go as single-chunk pieces so the first
            # projection chain can start sooner; the rest as pairs.
            pieces = [(0, 1), (1, 1), (2, 2), (4, 2), (6, 2), (8, 2), (10, 2),
                      (12, 2), (14, 2)]
            for lo, n in pieces:
                nc.sync.dma_start(xb0[:, lo:lo + n, :], xt0[:, lo:lo + n, :])
                nc.scalar.dma_start(w_q[:, lo:lo + n, :], wq_r[:, lo:lo + n, :])
            w_k = const.tile([P, HC, GO], BF16, tag="wk")
            nc.sync.dma_start(w_k[:], wkt[:].rearrange("(hc p) o -> p hc o", p=P))
            cos_t = const.tile([P, S], BF16, tag="cos")
            nc.scalar.dma_start(cos_t[:], cost[:])
            sin_t = const.tile([P, S], BF16, tag="sin")
            nc.scalar.dma_start(sin_t[:], sint[:])
            mask_t = const.tile([P, P], BF16, tag="mask")
            nc.scalar.dma_start(mask_t[:], maskt[:])
            w_v = const.tile([P, HC, GO], BF16, tag="wv")
            nc.sync.dma_start(w_v[:], wvt[:].rearrange("(hc p) o -> p hc o", p=P))
            xbs = [xb0]
            xb1 = xpool.tile([P, HC, SB], BF16, tag="xb")
            nc.sync.dma_start(xb1[:], xt_r[:, :, ts(1, SB)])
            xbs.append(xb1)

            q_t = const.tile([P, GH, S], BF16, tag="qt")
            k_t = const.tile([P, GH, S], BF16, tag="kt")
            v_t = const.tile([P, NKC, GO], BF16, tag="vt")
            out_t = const.tile([P, GH, S], BF16, tag="ot")
            # memset on DVE, not gpsimd: an otherwise-unused gpsimd engine
            # responds to the end-of-kernel barrier ~3.4us late (slow
            # software semaphore polling), stretching every exec.
            ones_t = const.tile([P, P], BF16, tag="ones")
            nc.vector.memset(ones_t[:], 1.0)

            MM_NS = 0.4167e-3  # PE stream: us per column

            # ---- emission generators.  Every yield is a CLOSED PE group
            # boundary; the yielded value is the quantum's PE time (us). ----

            def proj_sb(sb, xb):
                """One token-block of Q/K (with RoPE) and V projections."""
                for w_t, dest in ((w_q, q_t), (w_k, k_t)):
                    for h in range(GH):
                        ps = pacc.tile([P, SB], F32, tag="pp")
                        for hc in range(HC):
                            nc.tensor.matmul(
                                ps[:], w_t[:, hc, ts(h, P)], xb[:, hc, :],
                                start=(hc == 0), stop=(hc == HC - 1),
                            )
                        raw = dest[:, h, ts(sb, SB)]
                        nc.scalar.copy(raw, ps[:])
                        # RoPE: rot = raw*cos + shift(raw)*sin_signed
                        tmp = rpool.tile([P, SB], BF16, tag="sh")
                        nc.sync.dma_start(tmp[0:64, :], raw[64:128, :])
                        nc.sync.dma_start(tmp[64:128, :], raw[0:64, :])
                        tmp2 = rpool.tile([P, SB], BF16, tag="sp")
                        nc.vector.tensor_mul(tmp2[:], tmp[:], sin_t[:, ts(sb, SB)])
                        nc.vector.tensor_mul(raw, raw, cos_t[:, ts(sb, SB)])
                        nc.vector.tensor_add(raw, raw, tmp2[:])
                        yield 3.41
                for j in range(SB // P):
                    ps = pacc.tile([P, GO], F32, tag="pp")
                    for hc in range(HC):
                        nc.tensor.matmul(
                            ps[:], xb[:, hc, ts(j, P)], w_v[:, hc, :],
                            start=(hc == 0), stop=(hc == HC - 1),
                        )
                    nc.scalar.copy(v_t[:, sb * (SB // P) + j, :], ps[:])
                    yield 3.41

            def attn_block(b):
                """Attention for one 512-query block.

                Per head: score matmuls as closed start|stop groups (chunk
                pairs share a 2-bank psum tile and one paired ACT exp), the
                denominator accumulating on DVE behind the exps, then one
                uninterrupted PV accumulation chain.

                For the small blocks (b<2) several heads pack into one `at`
                tile so ALL scores run before the first PV -- the exps get
                multiple score-groups of slack.  Big blocks pipeline PV one
                head behind the scores."""
                nchunks = 4 * (b + 1)
                noff = 4 * b
                hpt = {0: 4, 1: 2}.get(b, 1)   # heads per at tile
                pend = []                       # (h, at, base, den1)

                def emit_pv(h, at, base, den1, last=False):
                    po = pout.tile([P, SB], F32, tag="po")
                    for c in range(nchunks):
                        j = c - noff
                        qlo = 128 * j if j > 0 else 0
                        nc.tensor.matmul(
                            po[:, qlo:], v_t[:, c, ts(h, P)],
                            at[:, base + c, qlo:],
                            start=(c == 0), stop=(c == nchunks - 1),
                        )
                    # denominator: one 512-col all-ones matmul sums the DVE
                    # chunk-fold over the 128 key partitions AND broadcasts
                    # it to all partitions (cost is per-column, M-free).
                    # Emitted after the PV chain so the DVE fold has a full
                    # chain's slack before the PE reaches it.  The last pop
                    # of the last block borrows the (then idle) score pool
                    # so the first outproj(3) psum tile doesn't queue behind
                    # the denominator's DVE read.
                    if last and b == NSB - 1:
                        psd = pscore.tile([P, 2, SB], F32, tag="ps")
                        pd = psd[:, 0, :]
                    else:
                        pdt = pacc.tile([P, SB], F32, tag="pp")
                        pd = pdt[:]
                    nc.tensor.matmul(pd, ones_t[:], den1, start=True, stop=True)
                    rec = dpool.tile([P, SB], F32, tag="rec")
                    nc.vector.reciprocal_approx_fast(rec[:], pd)
                    nc.vector.tensor_mul(out_t[:, h, ts(b, SB)], po[:], rec[:])

                at = den = None
                for h in range(GH):
                    if h % hpt == 0:
                        at = apool.tile([P, NKC, SB], BF16, tag="at")
                    if h % 2 == 0 and b < 2:
                        den = dpool.tile([P, 2, SB], BF16, tag="den")
                    elif b >= 2:
                        den = dpool.tile([P, 2, SB], BF16, tag="den")
                    base = (h % hpt) * nchunks
                    # b<2: each head owns ONE half of a shared den tile;
                    # b>=2: a full tile (pairs use both halves, then fold).
                    den1 = den[:, h % 2, :] if b < 2 else den[:, 0, :]
                    # off-diagonal chunk pairs
                    for i in range(noff // 2):
                        psc = pscore.tile([P, 2, SB], F32, tag="ps")
                        for m in (0, 1):
                            c = 2 * i + m
                            nc.tensor.matmul(
                                psc[:, m, :], k_t[:, h, ts(c, P)],
                                q_t[:, h, ts(b, SB)],
                                start=True, stop=True,
                            )
                        s = base + 2 * i
                        nc.scalar.activation(
                            at[:, s:s + 2, :], psc[:, :, :], EXP, scale=SCALE
                        )
                        if b >= 2:
                            if i == 0:
                                nc.vector.tensor_copy(den[:], at[:, s:s + 2, :])
                            else:
                                nc.vector.tensor_add(
                                    den[:], den[:], at[:, s:s + 2, :]
                                )
                        elif i == 0:
                            nc.vector.tensor_add(
                                den1, at[:, s, :], at[:, s + 1, :]
                            )
                        else:
                            nc.vector.tensor_add(den1, den1, at[:, s, :])
                            nc.vector.tensor_add(den1, den1, at[:, s + 1, :])
                        yield 0.43
                    # diagonal chunks, two per psum pair tile; the causal
                    # triangle is zeroed by a DVE mask multiply after exp
                    for i in range(2):
                        psc = pscore.tile([P, 2, SB], F32, tag="ps")
                        for m in (0, 1):
                            j = 2 * i + m
                            qlo = 128 * j
                            n = SB - qlo
                            nc.tensor.matmul(
                                psc[:, m, 0:n], k_t[:, h, ts(noff + j, P)],
                                q_t[:, h, b * SB + qlo:(b + 1) * SB],
                                start=True, stop=True,
                            )
                        for m in (0, 1):
                            j = 2 * i + m
                            qlo = 128 * j
                            n = SB - qlo
                            s = base + noff + j
                            nc.scalar.activation(
                                at[:, s, qlo:], psc[:, m, 0:n], EXP,
                                scale=SCALE,
                            )
                            nc.vector.tensor_mul(
                                at[:, s, qlo:qlo + P],
                                at[:, s, qlo:qlo + P], mask_t[:],
                            )
                            if b == 0 and j == 0:
                                nc.vector.tensor_copy(den1, at[:, s, :])
                            else:
                                dst = den1 if b < 2 else den[:, 0, :]
                                nc.vector.tensor_add(
                                    dst[:, qlo:] if b < 2 else den[:, 0, qlo:],
                                    dst[:, qlo:] if b < 2 else den[:, 0, qlo:],
                                    at[:, s, qlo:],
                                )
                        yield 0.37
                    if b >= 2:
                        # fold the two accumulator halves on DVE; overlaps
                        # the next head's scores so the PE den matmul in
                        # emit_pv never waits on it.
                        nc.vector.tensor_add(
                            den[:, 0, :], den[:, 0, :], den[:, 1, :]
                        )
                        den1 = den[:, 0, :]
                    pend.append((h, at, base, den1))
                    while len(pend) > (GH if b < 2 else 1):
                        emit_pv(*pend.pop(0))
                        yield (1280 + 2048 * b) * MM_NS
                while pend:
                    emit_pv(*pend.pop(0), last=(not pend))
                    yield (1280 + 2048 * b) * MM_NS

            NT = H // P
            OQ = (nc.sync, nc.scalar)

            def outproj_block(ob, tail=False):
                """Output projection of one query block; yields per closed
                4-matmul chain.  Output DMAs rotate across the sync/scalar
                queues.  While interleaved with attention the PSUM->SBUF
                staging copies stay on DVE (keeping ACT free for the
                latency-critical exps); in the tail they alternate, and the
                last chains split the copy across ACT/DVE halves with two
                64KB DMAs so the post-matmul drain is short."""
                for nt in range(NT):
                    if tail and nt == NT - 1:
                        # final chain: two 256-col sub-chains so the last
                        # copy+DMA starts half a chain earlier
                        HB = SB // 2
                        for half in range(2):
                            co = ob * SB + half * HB
                            pyt = pacc.tile([P, SB], F32, tag="pp")
                            for oc in range(GH):
                                nc.tensor.matmul(
                                    pyt[:, 0:HB], w_o[:, oc, ts(nt, P)],
                                    out_t[:, oc, co:co + HB],
                                    start=(oc == 0), stop=(oc == GH - 1),
                                )
                            ysb = ypool.tile([P, SB], BF16, tag="ysb")
                            if half == 0:
                                nc.scalar.copy(ysb[:, 0:HB], pyt[:, 0:HB])
                            else:
                                nc.vector.tensor_copy(ysb[:, 0:HB],
                                                      pyt[:, 0:HB])
                            OQ[half].dma_start(
                                yt_r[:, nt, co:co + HB], ysb[:, 0:HB]
                            )
                        yield 0.85
                        continue
                    pyt = pacc.tile([P, SB], F32, tag="pp")
                    for oc in range(GH):
                        nc.tensor.matmul(
                            pyt[:], w_o[:, oc, ts(nt, P)],
                            out_t[:, oc, ts(ob, SB)],
                            start=(oc == 0), stop=(oc == GH - 1),
                        )
                    ysb = ypool.tile([P, SB], BF16, tag="ysb")
                    if tail and nt >= NT - 4:
                        HB = SB // 2
                        nc.scalar.copy(ysb[:, 0:HB], pyt[:, 0:HB])
                        nc.vector.tensor_copy(ysb[:, HB:SB], pyt[:, HB:SB])
                        OQ[nt % 2].dma_start(
                            yt_r[:, nt, ob * SB:ob * SB + HB], ysb[:, 0:HB]
                        )
                        OQ[(nt + 1) % 2].dma_start(
                            yt_r[:, nt, ob * SB + HB:(ob + 1) * SB],
                            ysb[:, HB:SB],
                        )
                    else:
                        if tail and nt % 2 == 1:
                            nc.scalar.copy(ysb[:], pyt[:])
                        else:
                            nc.vector.tensor_copy(ysb[:], pyt[:])
                        OQ[nt % 2].dma_start(yt_r[:, nt, ts(ob, SB)], ysb[:])
                    yield 0.85

            def drain(gen):
                for _ in gen:
                    pass

            def chain2(*gens):
                for g in gens:
                    yield from g

            def interleave(primary, filler, ratio, drain_rest=True):
                """Emit primary; between its quanta emit filler quanta so
                filler-PE-time ~= ratio * primary-PE-time."""
                acc = 0.0
                done = False
                for wp in primary:
                    acc += ratio * (wp or 1.0)
                    while not done and acc > 0:
                        wf = next(filler, None)
                        if wf is None:
                            done = True
                        else:
                            acc -= wf or 1.0
                if drain_rest and not done:
                    drain(filler)

            # ---- drive ----
            drain(proj_sb(0, xbs[0]))

            def proj_one(sb):
                # prefetch the NEXT x block after the first chain of this
                # one, so its (single-queue) DMA lands well before use.
                first = True
                for w in proj_sb(sb, xbs[sb]):
                    yield w
                    if first:
                        first = False
                        if sb + 1 < NSB and len(xbs) == sb + 1:
                            xbn = xpool.tile([P, HC, SB], BF16, tag="xb")
                            nc.sync.dma_start(xbn[:], xt_r[:, :, ts(sb + 1, SB)])
                            xbs.append(xbn)

            interleave(proj_one(1), attn_block(0), 5.2 / 41.0)
            interleave(proj_one(2), attn_block(1), 12.1 / 41.0)
            # Wo reuses an x-stream slot (same bytes); loaded once proj(2)
            # has consumed xb2, well before outproj(0) needs it.
            w_o = xpool.tile([P, GH, H], BF16, tag="xb")
            nc.sync.dma_start(w_o[:], wot[:].rearrange("(oc p) n -> p oc n", p=P))
            interleave(proj_one(3), attn_block(2), 18.8 / 41.0)
            # ratio slightly under-fills so a couple of outproj(2) chains
            # remain after attn(3) to cover its DVE normalize tail
            interleave(
                attn_block(3),
                chain2(outproj_block(0), outproj_block(1), outproj_block(2)),
                1.5,
            )
            drain(outproj_block(NSB - 1, tail=True))

    nc.compile()
    return nc


def _get_nc():
    if "nc" not in _built:
        _built["nc"] = _build()
    return _built["nc"]


def _host_inputs(x, Wq, Wk, Wv, Wo):
    bf = ml_dtypes.bfloat16
    inv = 1.0 / (10000.0 ** (np.arange(0, HD, 2, dtype=np.float64) / HD))
    t = np.arange(S, dtype=np.float64)
    fr = np.outer(t, inv)                       # [S, 64]
    cos = np.cos(fr)
    sin = np.sin(fr)
    cosT = np.concatenate([cos, cos], axis=1).T.astype(bf)      # [128, S]
    sinT = np.concatenate([-sin, sin], axis=1).T.astype(bf)     # signed
    a = np.arange(P)
    mask01 = (a[:, None] <= a[None, :]).astype(bf)

    in_maps = []
    for core in range(NCORES):
        b, g = divmod(core, GH)
        sl = slice(GO * g, GO * (g + 1))
        in_maps.append({
            "xt": np.ascontiguousarray(x[b].T).astype(bf),
            "wqt": np.ascontiguousarray(Wq[sl, :].T).astype(bf),
            "wkt": np.ascontiguousarray(Wk[sl, :].T).astype(bf),
            "wvt": np.ascontiguousarray(Wv[sl, :].T).astype(bf),
            "wot": np.ascontiguousarray(Wo[:, sl].T).astype(bf),
            "cost": cosT.copy(),
            "sint": sinT.copy(),
            "maskt": mask01.copy(),
        })
    return in_maps


def kernel(x, Wq, Wk, Wv, Wo, _trace=False):
    x = np.asarray(x, dtype=np.float32)
    Wq = np.asarray(Wq, dtype=np.float32)
    Wk = np.asarray(Wk, dtype=np.float32)
    Wv = np.asarray(Wv, dtype=np.float32)
    Wo = np.asarray(Wo, dtype=np.float32)

    nc = _get_nc()
    in_maps = _host_inputs(x, Wq, Wk, Wv, Wo)
    res = run_bass_kernel_spmd(
        nc, in_maps, core_ids=list(range(NCORES)), trace=_trace
    )
    if _trace:
        _built["last_result"] = res

    y = np.zeros((B, S, H), dtype=np.float32)
    for core in range(NCORES):
        b = core // GH
        y[b] += res.results[core]["yt"].T.astype(np.float32)
    return y
